# revision 15
# baseline (speedup 1.0000x reference)
"""Trainium2 Bass kernel for the 4-layer ARMAConv GNN (nn_Net_52587579572464).

Math (per graph, per layer, K=3 stacks):
    h_out = relu(mean_k relu(a @ (x @ W_k) + x @ V_k + b_k))
Restructured:
    xa = a @ x                      (shared across the K stacks: a(xW) == (ax)W)
    Z  = [x ; xa] @ [[V_k];[W_k]]   (one matmul, 3 stacks stacked into columns)
    h_out = sum_k relu(Z_k + b_k)   (outer relu is a no-op on a sum of relus;
                                     the 1/3 mean is folded into the next
                                     layer's weights / the dense head)

Device mapping per NeuronCore (16 graphs each, 8 cores data-parallel):
    - all matmul operands bf16 (1 cycle/row incl. transposes), fp32 PSUM
    - ALL weights resident in SBUF (~93 KB/partition in bf16) — loaded once,
      no per-pair re-streaming from HBM
    - activations kept feature-major (hT: [C,400]) for the channel matmul;
      node-major copy (h: [512pad,C]) via PE transposes for the GSO matmul
    - chan matmul f-outer / graph-inner so each stationary U block serves
      both graphs of the pair back-to-back (halves LDWEIGHTS traffic)
"""

import sys

for _p in ("/opt/trn_rl_repo", "/root/.axon_site/_ro/trn_rl_repo"):
    if _p not in sys.path:
        sys.path.insert(0, _p)

from contextlib import ExitStack, nullcontext

import numpy as np

import concourse.bass as bass
import concourse.bacc as bacc
import concourse.tile as tile
from concourse import mybir
from concourse.masks import make_identity

F32 = mybir.dt.float32
BF16 = mybir.dt.bfloat16
RELU = mybir.ActivationFunctionType.Relu

NCORES = 8
B = 128
G = B // NCORES          # graphs per core
N = 400                  # nodes
NP = 512                 # nodes padded (contraction dim of the GSO matmul)
F = 240                  # input features
FP = 256                 # input features padded
C = 512                  # hidden channels
K = 3                    # ARMA stacks
L = 480                  # labels
NMT = NP // 128          # 4 node m-tiles
NCC = C // 128           # 4 channel chunks
NJ = K * NCC             # 12 cout blocks of 128
NFS = (4, 8, 8, 8)       # contraction f-tiles per layer


def _dedupe_ldweights(nc):
    """Remove InstLdweights identical to the previous PE weight load in the
    same block with no intervening clobber (self-loading matmul or other PE
    instruction). The paired InstMatmult already has ldweights=False and
    simply reuses the stationary operand still resident in the PE array.
    Measured on HW: numerically exact, saves the redundant weight-load slot.
    """
    pe = mybir.EngineType.PE
    removed = 0
    for b in nc.m.functions[0].blocks:
        prev_key = None
        dead = set()
        for i in b.instructions:
            if i.engine != pe:
                continue
            if isinstance(i, mybir.InstLdweights):
                a = i.ins[0]
                key = (str(a.memref), a.offset, str(a.ap), str(a.dtype),
                       i.is_transpose, str(i.perf_mode), i.tile_position)
                si = i.sync_info
                has_sync = si is not None and (
                    len(si.on_wait) or len(si.on_update)
                )
                if key == prev_key and not has_sync:
                    dead.add(i.name)
                else:
                    prev_key = key
            elif isinstance(i, mybir.InstMatmult):
                if i.ldweights:
                    prev_key = None
            elif isinstance(i, mybir.InstEventSemaphore):
                pass
            else:
                prev_key = None
        if dead:
            insts = [x for x in b.instructions if x.name not in dead]
            b.instructions.clear()
            b.instructions.extend(insts)
            removed += len(dead)
    return removed


def _strip_unwaited_mm_updates(nc):
    """Every matmul increments the Tile PE progress semaphore (~26 ns of
    serialized EVT_SEM register traffic each), but only the increments whose
    cumulative count appears as a wait threshold are observable. Strip the
    rest and renumber all wait thresholds exactly.
    """
    f = nc.m.functions[0]
    pe = mybir.EngineType.PE

    # the PE progress semaphore: the one PE matmuls inc
    from collections import Counter
    sem_ids = Counter()
    for b in f.blocks:
        for i in b.instructions:
            if i.engine != pe or not isinstance(i, mybir.InstMatmult):
                continue
            si = i.sync_info
            if si is None:
                continue
            for u in si.on_update:
                if u.update_mode == "sem-inc" and u.update_value == 1:
                    sem_ids[u.id] += 1
    if not sem_ids:
        return 0
    SEM = sem_ids.most_common(1)[0][0]

    awaited = set()
    for b in f.blocks:
        for i in b.instructions:
            si = i.sync_info
            if si is None:
                continue
            for w in si.on_wait:
                if w.id == SEM:
                    assert w.wait_mode == "sem-ge-imm", w.wait_mode
                    awaited.add(w.wait_value)

    # walk PE stream in order: cumulative count, decide keep/strip
    cum = 0
    stripped_before = {}  # old count -> number stripped among first `count` incs
    nstrip = 0
    strip_insts = []
    for b in f.blocks:
        for i in b.instructions:
            if i.engine != pe:
                continue
            si = i.sync_info
            if si is None:
                continue
            ups = [u for u in si.on_update if u.id == SEM]
            if not ups:
                continue
            assert len(ups) == 1 and isinstance(i, mybir.InstMatmult), i.name
            cum += 1
            if cum not in awaited:
                nstrip += 1
                strip_insts.append(i)
            stripped_before[cum] = nstrip

    if not nstrip:
        return 0
    for i in strip_insts:
        si = i.sync_info
        si.on_update = [u for u in si.on_update if u.id != SEM]

    # renumber wait thresholds
    for b in f.blocks:
        for i in b.instructions:
            si = i.sync_info
            if si is None:
                continue
            for w in si.on_wait:
                if w.id == SEM:
                    w.wait_value = w.wait_value - stripped_before[w.wait_value]
    return nstrip


def _build_nc(reps=1):
    nc = bacc.Bacc("TRN2", target_bir_lowering=False)

    xt_d = nc.dram_tensor("xt", [G, 128, 2, N], BF16, kind="ExternalInput")
    xn_d = nc.dram_tensor("xn", [G, 128, 3, 2, 128], BF16, kind="ExternalInput")
    xnt_d = nc.dram_tensor("xnt", [G, 128, 128], BF16, kind="ExternalInput")
    at_d = nc.dram_tensor("at", [128, 3, N], BF16, kind="ExternalInput")
    att_d = nc.dram_tensor("att", [128, N], BF16, kind="ExternalInput")
    u_ds = [
        nc.dram_tensor(f"u{l}", [128, NJ, NFS[l - 1], 128], BF16,
                       kind="ExternalInput")
        for l in (1, 2, 3, 4)
    ]
    wdt_d = nc.dram_tensor("wdt", [128, NCC, 4, 128], BF16, kind="ExternalInput")
    bias_d = nc.dram_tensor("bias", [128, 4, NJ], F32, kind="ExternalInput")
    bdt_d = nc.dram_tensor("bdt", [128, 4], F32, kind="ExternalInput")
    # label-major output: y_d[g, p, lc, n] = y[g, n, lc*128+p]
    y_d = nc.dram_tensor("y", [G, 128, 4, N], F32, kind="ExternalOutput")

    with tile.TileContext(nc) as tc, ExitStack() as ctx:
        const = ctx.enter_context(tc.tile_pool(name="const", bufs=1))
        xpool = ctx.enter_context(tc.tile_pool(name="xpool", bufs=2))
        hpool = ctx.enter_context(tc.tile_pool(name="hpool", bufs=2))
        rpool = ctx.enter_context(tc.tile_pool(name="rpool", bufs=2))
        ypool = ctx.enter_context(tc.tile_pool(name="ypool", bufs=2))
        ps_gso = ctx.enter_context(tc.tile_pool(name="ps_gso", bufs=2, space="PSUM"))
        ps_chan = ctx.enter_context(tc.tile_pool(name="ps_chan", bufs=5, space="PSUM"))
        ps_tr = ctx.enter_context(tc.tile_pool(name="ps_tr", bufs=1, space="PSUM"))

        # ---- resident constants (weights loaded once, never re-streamed).
        # Only what layer 1 needs is DMA'd up front; the bulk U2-4 loads are
        # issued inside pair 0's body so they don't head-of-line-block the
        # first pair's inputs on the SP DMA queue. ----
        ident = const.tile([128, 128], BF16)
        make_identity(nc, ident)
        u_sbs = [
            const.tile([128, NJ, NFS[l - 1], 128], BF16, tag=f"u{l}",
                       name=f"u_sb{l}")
            for l in (1, 2, 3, 4)
        ]
        at_sb = const.tile([128, 3, N], BF16)
        nc.scalar.dma_start(out=at_sb[:], in_=at_d[:])
        att_sb = const.tile([128, N], BF16)
        nc.scalar.dma_start(out=att_sb[:], in_=att_d[:])
        # u1/bias/wdt/bdt are DMA'd from inside pair 0's body, after its
        # input loads, ordered by first use so the first GSO isn't queued
        # behind them.
        wdt_sb = const.tile([128, NCC, 4, 128], BF16)
        bias_sb = const.tile([128, 4, NJ], F32)
        bdt_sb = const.tile([128, 4], F32)

        def gso_layer1(g, xn_g, xnt_g, xa_out):
            # Layer-1 GSO (2 feature chunks): 3 full m-tiles + packed tail.
            # The tails of both fc chunks sit at partition offsets 0/32 of
            # xnt, so the two row-tiled tail matmuls co-execute on the PE.
            pss = {}
            for fc in (0, 1):
                ps = ps_gso.tile([128, N], F32, tag="gso", name=f"psg1_{g}_{fc}")
                for mt in range(3):
                    nc.tensor.matmul(
                        ps[:],
                        xn_g[:, mt, fc, :],
                        at_sb[:, mt, :],
                        start=(mt == 0),
                        stop=False,
                    )
                pss[fc] = ps
            for fc in (0, 1):
                nc.tensor.matmul(
                    pss[fc][:],
                    xnt_g[32 * fc : 32 * fc + 16, :],
                    att_sb[32 * fc : 32 * fc + 16, :],
                    start=False,
                    stop=True,
                    tile_position=(32 * fc, 0),
                )
            for fc in (0, 1):
                nc.vector.tensor_copy(xa_out[:, fc, :], pss[fc][:])

        def gso_layer2(g, li, hn_g, hnt_g, xa_out):
            # Layers 2-4 GSO: 3 full 128-node m-tiles + a packed tail step.
            # The tail (nodes 384:399, 16 rows) for both fc of a half sits at
            # 32-aligned partition offsets in hn_tail, so the two row-tiled
            # matmuls occupy different PE row groups and co-execute.
            for half in (0, 1):
                fcs = (2 * half, 2 * half + 1)
                pss = {}
                for fc in fcs:
                    ps = ps_gso.tile([128, N], F32, tag="gso",
                                     name=f"psg_{g}_{li}_{fc}")
                    for mt in range(3):
                        nc.tensor.matmul(
                            ps[:],
                            hn_g[:, fc, mt, :],
                            at_sb[:, mt, :],
                            start=(mt == 0),
                            stop=False,
                        )
                    pss[fc] = ps
                for fc in fcs:
                    nc.tensor.matmul(
                        pss[fc][:],
                        hnt_g[32 * fc : 32 * fc + 16, :],
                        att_sb[32 * fc : 32 * fc + 16, :],
                        start=False,
                        stop=True,
                        tile_position=(32 * fc, 0),
                    )
                for fc in fcs:
                    nc.vector.tensor_copy(xa_out[:, fc, :], pss[fc][:])

        def chan_layer(gs, li, u_sb, nf, moving, hT_out):
            # Z.T[jj] = sum_f U[f,jj].T @ xcatT[f] ; relu+bias ; sum K stacks.
            # f outer / graph inner: each stationary U block serves both
            # graphs of the pair back-to-back.
            for cc in range(NCC):
                rk = {g: [] for g in gs}
                for k in range(K):
                    jj = k * NCC + cc
                    pss = {
                        g: ps_chan.tile([128, 512], F32, tag="chan",
                                        name=f"psc_{g}_{jj}")
                        for g in gs
                    }
                    for f in range(nf):
                        for g in gs:
                            nc.tensor.matmul(
                                pss[g][:, :N],
                                u_sb[:, jj, f, :],
                                moving(g, f),
                                start=(f == 0),
                                stop=(f == nf - 1),
                            )
                    for g in gs:
                        r = rpool.tile([128, N], BF16, tag=f"r{g % 2}_{k}",
                                       name=f"r_{g}_{k}")
                        nc.scalar.activation(
                            r[:], pss[g][:, :N], RELU,
                            bias=bias_sb[:, li, jj : jj + 1],
                        )
                        rk[g].append(r)
                for g in gs:
                    nc.vector.tensor_add(hT_out[g][:, cc, :N], rk[g][0][:], rk[g][1][:])
                    nc.vector.tensor_add(
                        hT_out[g][:, cc, :N], hT_out[g][:, cc, :N], rk[g][2][:]
                    )

        def transpose_h(hT, h_node, h_tail):
            # hT [128, NCC, 512] -> h_node [128, NCC, 3, 128] (m-tiles 0-2)
            # plus h_tail [128, 128]: partition 32*cc+i = node 384+i of chunk
            # cc (i>=16 is junk from hT cols 400:415, never read).
            # The 12 full 128x128 transposes go through the DMA xbar
            # transpose (idle DMA engines, ~112 ns each) instead of the PE;
            # only the 4 cheap 16-col tail transposes stay on the PE (they
            # need the 32-aligned partition packing for GSO co-execution).
            ps = ps_tr.tile([128, 128], BF16, tag="tr")
            for cc in range(NCC):
                nc.tensor.transpose(
                    ps[32 * cc : 32 * cc + 16, :],
                    hT[:, cc, 384:400],
                    ident[:],
                    tile_position=(0, 32 * cc),
                )
            nc.vector.tensor_copy(h_tail[:], ps[:])
            for cc in range(NCC):
                for mt in range(3):
                    nc.sync.dma_start_transpose(
                        out=h_node[:, cc, mt, :],
                        in_=hT[:, cc, mt * 128 : (mt + 1) * 128],
                    )

        rep_ctx = tc.For_i(0, reps, 1) if reps > 1 else nullcontext()
        with rep_ctx:
         for pp in range(G // 2):
            gs = (2 * pp, 2 * pp + 1)

            # ---------- layer 1 ----------
            xt_sb, xn_sb, xnt_sb, xa1, hT, hn, hnt = {}, {}, {}, {}, {}, {}, {}
            for g in gs:
                # xn before xt: the first GSO needs only at+xn; xt is first
                # read by the chan layer a few us later.
                xn_sb[g] = xpool.tile([128, 3, 2, 128], BF16, tag=f"xn{g % 2}",
                                      name=f"xn_g{g}")
                nc.sync.dma_start(out=xn_sb[g][:], in_=xn_d[g])
                xnt_sb[g] = xpool.tile([128, 128], BF16, tag=f"xnt{g % 2}",
                                       name=f"xnt_g{g}")
                nc.sync.dma_start(out=xnt_sb[g][:], in_=xnt_d[g])
                xt_sb[g] = xpool.tile([128, 2, N], BF16, tag=f"xt{g % 2}",
                                      name=f"xt_g{g}")
                nc.sync.dma_start(out=xt_sb[g][:], in_=xt_d[g])
                xa1[g] = xpool.tile([128, 2, N], BF16, tag=f"xa_{g % 2}",
                                    name=f"xa1_g{g}")
                gso_layer1(g, xn_sb[g], xnt_sb[g], xa1[g])
                hT[g] = hpool.tile([128, NCC, 512], BF16, tag=f"hT{g % 2}",
                                   name=f"hT1_g{g}")
            if pp == 0:
                nc.sync.dma_start(out=u_sbs[0][:], in_=u_ds[0][:])
                nc.sync.dma_start(out=bias_sb[:], in_=bias_d[:])
            chan_layer(
                gs, 0, u_sbs[0], 4,
                lambda g, f: xt_sb[g][:, f, :] if f < 2 else xa1[g][:, f - 2, :],
                hT,
            )
            if pp == 0:
                # Bulk loads issued after pair 0's L1 so they don't
                # head-of-line block the first pair's inputs on the DMA queue.
                for l in (1, 2, 3):
                    nc.sync.dma_start(out=u_sbs[l][:], in_=u_ds[l][:])
                nc.sync.dma_start(out=wdt_sb[:], in_=wdt_d[:])
                nc.sync.dma_start(out=bdt_sb[:], in_=bdt_d[:])
            for g in gs:
                hn[g] = hpool.tile([128, NCC, 3, 128], BF16, tag=f"hn{g % 2}",
                                   name=f"hn1_g{g}")
                hnt[g] = hpool.tile([128, 128], BF16, tag=f"hnt{g % 2}",
                                    name=f"hnt1_g{g}")
                transpose_h(hT[g], hn[g], hnt[g])

            # ---------- layers 2..4 ----------
            for li, u_sb in zip((1, 2, 3), u_sbs[1:]):
                xa, hT_new, hn_new, hnt_new = {}, {}, {}, {}
                for g in gs:
                    xa[g] = xpool.tile([128, NCC, N], BF16, tag=f"xa_{g % 2}",
                                       name=f"xa_g{g}_l{li}")
                    gso_layer2(g, li, hn[g], hnt[g], xa[g])
                    hT_new[g] = hpool.tile([128, NCC, 512], BF16, tag=f"hT{g % 2}",
                                           name=f"hT_g{g}_l{li}")
                chan_layer(
                    gs, li, u_sb, 8,
                    lambda g, f: hT[g][:, f, :N] if f < NCC else xa[g][:, f - NCC, :],
                    hT_new,
                )
                if li < 3:
                    for g in gs:
                        hn_new[g] = hpool.tile([128, NCC, 3, 128], BF16,
                                               tag=f"hn{g % 2}", name=f"hn_g{g}_l{li}")
                        hnt_new[g] = hpool.tile([128, 128], BF16,
                                                tag=f"hnt{g % 2}",
                                                name=f"hnt_g{g}_l{li}")
                        transpose_h(hT_new[g], hn_new[g], hnt_new[g])
                    hn, hnt = hn_new, hnt_new
                hT = hT_new

            # ---------- dense head (label-major) ----------
            # yT[lc*128+p, n] = sum_c Wd.T[l, c] h[n, c] / 3 + bd[l].
            # Stationary = Wd chunk (shared by both graphs of the pair and
            # LDW-deduped), moving = hT chunk (400 wide vs 480 node-major).
            # Host un-packs the label-major layout.
            y_sb = {}
            for g in gs:
                y_sb[g] = ypool.tile([128, 4, N], F32, tag=f"y{g % 2}",
                                     name=f"y_{g}")
            for lc in range(4):
                pss = {}
                for cc in range(NCC):
                    for g in gs:
                        if cc == 0:
                            pss[g] = ps_chan.tile([128, 512], F32, tag="chan",
                                                  name=f"psy_{g}_{lc}")
                        nc.tensor.matmul(
                            pss[g][:, :N],
                            wdt_sb[:, cc, lc, :],
                            hT[g][:, cc, :N],
                            start=(cc == 0),
                            stop=(cc == NCC - 1),
                        )
                for g in gs:
                    nc.vector.tensor_scalar_add(
                        y_sb[g][:, lc, :], pss[g][:, :N],
                        bdt_sb[:, lc : lc + 1],
                    )
            for g in gs:
                nc.scalar.dma_start(out=y_d[g], in_=y_sb[g][:])

    nc.compile()
    if reps == 1:
        # straight-line code only: the semaphore renumbering in the strip
        # pass assumes each block executes once
        _dedupe_ldweights(nc)
        _strip_unwaited_mm_updates(nc)
    return nc


def _pack_inputs(x, a, Ws, Vs, bs, Wd, bd):
    """Host-side packing into the per-core DMA-friendly layouts (bf16)."""
    import ml_dtypes

    BF = ml_dtypes.bfloat16
    x = np.asarray(x, np.float32)
    a = np.asarray(a, np.float32)

    # aT m-tiles 0-2: [p, mt, n] = a[n, mt*128+p] (tail nodes live in att)
    at_pack = np.ascontiguousarray(
        a.T[: 3 * 128, :].reshape(3, 128, N).transpose(1, 0, 2)
    ).astype(BF)

    # tail replica: att[32*cc + i, n] = a[n, 384 + i] for i < 16, all 4 cc
    att_pack = np.zeros((128, N), np.float32)
    for cc in range(NCC):
        att_pack[32 * cc : 32 * cc + 16, :] = a.T[384:400, :]
    att_pack = att_pack.astype(BF)

    # U layouts: [128, NJ, nf, 128]; u[p, jj, f, c] = U[f*128+p, jj*128+c]
    # U1: rows [V1 pad 256 ; W1 pad 256], cols k-major (k*512 + c)
    U1 = np.zeros((512, K * C), np.float32)
    for k in range(K):
        U1[:F, k * C : (k + 1) * C] = Vs[0][k]
        U1[FP : FP + F, k * C : (k + 1) * C] = Ws[0][k]
    u_packs = [
        np.ascontiguousarray(
            U1.reshape(4, 128, NJ, 128).transpose(1, 2, 0, 3)
        ).astype(BF)
    ]
    for l in range(1, 4):
        U = np.empty((2 * C, K * C), np.float32)
        for k in range(K):
            U[:C, k * C : (k + 1) * C] = Vs[l][k] / 3.0
            U[C:, k * C : (k + 1) * C] = Ws[l][k] / 3.0
        u_packs.append(
            np.ascontiguousarray(
                U.reshape(8, 128, NJ, 128).transpose(1, 2, 0, 3)
            ).astype(BF)
        )

    # wdt[p, cc, lc, j] = Wd[cc*128+p, lc*128+j] / 3, zero-padded past L
    wd3 = np.asarray(Wd, np.float32) / 3.0
    wdt_pack = np.zeros((128, NCC, 4, 128), np.float32)
    for cc in range(NCC):
        for lc in range(4):
            w = min(128, L - lc * 128)
            wdt_pack[:, cc, lc, :w] = wd3[cc * 128 : (cc + 1) * 128,
                                          lc * 128 : lc * 128 + w]
    wdt_pack = wdt_pack.astype(BF)

    bias_pack = np.zeros((128, 4, NJ), np.float32)
    for li in range(4):
        for jj in range(NJ):
            k, cc = divmod(jj, NCC)
            bias_pack[:, li, jj] = bs[li][k, cc * 128 : (cc + 1) * 128]

    # bdt[p, lc] = bd[lc*128+p] (zero past L)
    bdt_pack = np.zeros((128, 4), np.float32)
    bdf = np.asarray(bd, np.float32)
    for lc in range(4):
        w = min(128, L - lc * 128)
        bdt_pack[:w, lc] = bdf[lc * 128 : lc * 128 + w]

    in_maps = []
    for c in range(NCORES):
        xs = x[c * G : (c + 1) * G]  # (G, 400, 240)
        x_pad = np.zeros((G, NP, FP), np.float32)
        x_pad[:, :N, :F] = xs
        xn_pack = np.ascontiguousarray(
            x_pad[:, : 3 * 128, :].reshape(G, 3, 128, 2, 128)
            .transpose(0, 2, 1, 3, 4)
        ).astype(BF)
        # xnt[g, 32*fc + i, c] = x[g, 384+i, fc*128+c] for fc in (0,1)
        xnt_pack = np.zeros((G, 128, 128), np.float32)
        for fc in (0, 1):
            xnt_pack[:, 32 * fc : 32 * fc + 16, :] = x_pad[
                :, 384:400, fc * 128 : (fc + 1) * 128
            ]
        xnt_pack = xnt_pack.astype(BF)
        xt_pack = np.ascontiguousarray(
            x_pad[:, :N, :].transpose(0, 2, 1).reshape(G, 2, 128, N).transpose(0, 2, 1, 3)
        ).astype(BF)
        in_maps.append(
            {
                "xt": xt_pack,
                "xn": xn_pack,
                "xnt": xnt_pack,
                "at": at_pack,
                "att": att_pack,
                "u1": u_packs[0],
                "u2": u_packs[1],
                "u3": u_packs[2],
                "u4": u_packs[3],
                "wdt": wdt_pack,
                "bias": bias_pack,
                "bdt": bdt_pack,
            }
        )
    return in_maps


_NC_CACHE = {}


def _get_nc(reps=1):
    key = f"nc{reps}"
    if key not in _NC_CACHE:
        _NC_CACHE[key] = _build_nc(reps)
    return _NC_CACHE[key]


def _get_runner():
    """Compile-once jitted SPMD executor (mirrors bass2jax.run_bass_via_pjrt's
    multi-core branch) so repeated kernel() calls skip recompilation."""
    if "runner" in _NC_CACHE:
        return _NC_CACHE["runner"]
    import jax
    import jax.numpy as jnp
    from jax.sharding import Mesh, PartitionSpec, NamedSharding
    from jax.experimental.shard_map import shard_map
    from concourse import bass2jax

    nc = _get_nc()
    bass2jax.install_neuronx_cc_hook()
    partition_name = nc.partition_id_tensor.name if nc.partition_id_tensor else None
    in_names, out_names, out_avals, zero_shapes = [], [], [], []
    for alloc in nc.m.functions[0].allocations:
        if not isinstance(alloc, mybir.MemoryLocationSet):
            continue
        name = alloc.memorylocations[0].name
        if alloc.kind == "ExternalInput":
            if name != partition_name:
                in_names.append(name)
        elif alloc.kind == "ExternalOutput":
            out_names.append(name)
            shape = tuple(alloc.tensor_shape)
            dtype = mybir.dt.np(alloc.dtype)
            out_avals.append(jax.core.ShapedArray(shape, dtype))
            zero_shapes.append((shape, dtype))
    n_params = len(in_names)
    n_outs = len(out_avals)
    all_names = list(in_names) + list(out_names)
    if partition_name is not None:
        all_names.append(partition_name)

    def _body(*args):
        operands = list(args)
        if partition_name is not None:
            operands.append(bass2jax.partition_id_tensor())
        outs = bass2jax._bass_exec_p.bind(
            *operands,
            out_avals=tuple(out_avals),
            in_names=tuple(all_names),
            out_names=tuple(out_names),
            lowering_input_output_aliases=(),
            sim_require_finite=True,
            sim_require_nnan=True,
            nc=nc,
        )
        return tuple(outs)

    devices = jax.devices()[:NCORES]
    mesh = Mesh(np.asarray(devices), ("core",))
    sharded = jax.jit(
        shard_map(
            _body,
            mesh=mesh,
            in_specs=(PartitionSpec("core"),) * (n_params + n_outs),
            out_specs=(PartitionSpec("core"),) * n_outs,
            check_rep=False,
        ),
        donate_argnums=tuple(range(n_params, n_params + n_outs)),
        keep_unused=True,
    )
    sh = NamedSharding(mesh, PartitionSpec("core"))
    make_zeros = jax.jit(
        lambda: tuple(
            jnp.zeros((NCORES * s[0], *s[1:]), d) for s, d in zero_shapes
        ),
        out_shardings=(sh,) * n_outs,
    )

    def run(in_maps):
        concat = [
            np.concatenate([np.asarray(m[name]) for m in in_maps], axis=0)
            for name in in_names
        ]
        dev_in = [jax.device_put(a, sh) for a in concat]
        outs = sharded(*dev_in, *make_zeros())
        oi = out_names.index("y")
        # y_raw[b, p, lc, n] = y[b, n, lc*128+p]
        y_raw = np.asarray(outs[oi])  # (B, 128, 4, N)
        y = np.ascontiguousarray(y_raw.transpose(0, 3, 2, 1)).reshape(B, N, 512)
        return y[:, :, :L]

    _NC_CACHE["runner"] = run
    return run


def kernel(
    x, a, W1, V1, b1, W2, V2, b2, W3, V3, b3, W4, V4, b4, Wd, bd
) -> np.ndarray:
    in_maps = _pack_inputs(
        x,
        a,
        [np.asarray(W, np.float32) for W in (W1, W2, W3, W4)],
        [np.asarray(V, np.float32) for V in (V1, V2, V3, V4)],
        [np.asarray(b, np.float32) for b in (b1, b2, b3, b4)],
        Wd,
        bd,
    )
    return _get_runner()(in_maps)



# revision 16
# speedup vs baseline: 1.2734x; 1.2734x over previous
"""Trainium2 Bass kernel for the 4-layer ARMAConv GNN (nn_Net_52587579572464).

Math (per graph, per layer, K=3 stacks):
    h_out = relu(mean_k relu(a @ (x @ W_k) + x @ V_k + b_k))
Restructured:
    xa = a @ x                      (shared across the K stacks: a(xW) == (ax)W)
    Z  = [x ; xa] @ [[V_k];[W_k]]   (one matmul, 3 stacks stacked into columns)
    h_out = sum_k relu(Z_k + b_k)   (outer relu is a no-op on a sum of relus;
                                     the 1/3 mean is folded into the next
                                     layer's weights / the dense head)

Device mapping per NeuronCore (16 graphs each, 8 cores data-parallel):
    - all matmul operands bf16 (1 cycle/row incl. transposes), fp32 PSUM
    - ALL weights resident in SBUF (~93 KB/partition in bf16) — loaded once,
      no per-pair re-streaming from HBM
    - activations kept feature-major (hT: [C,400]) for the channel matmul;
      node-major copy (h: [512pad,C]) via PE transposes for the GSO matmul
    - chan matmul f-outer / graph-inner so each stationary U block serves
      both graphs of the pair back-to-back (halves LDWEIGHTS traffic)
"""

import sys

for _p in ("/opt/trn_rl_repo", "/root/.axon_site/_ro/trn_rl_repo"):
    if _p not in sys.path:
        sys.path.insert(0, _p)

from contextlib import ExitStack, nullcontext

import numpy as np

import concourse.bass as bass
import concourse.bacc as bacc
import concourse.tile as tile
from concourse import mybir
from concourse.masks import make_identity

F32 = mybir.dt.float32
BF16 = mybir.dt.bfloat16
RELU = mybir.ActivationFunctionType.Relu

NCORES = 8
B = 128
G = B // NCORES          # graphs per core
N = 400                  # nodes
NP = 512                 # nodes padded (contraction dim of the GSO matmul)
F = 240                  # input features
FP = 256                 # input features padded
C = 512                  # hidden channels
K = 3                    # ARMA stacks
L = 480                  # labels
NMT = NP // 128          # 4 node m-tiles
NCC = C // 128           # 4 channel chunks
NJ = K * NCC             # 12 cout blocks of 128
NFS = (4, 8, 8, 8)       # contraction f-tiles per layer


def _dedupe_ldweights(nc):
    """Remove InstLdweights identical to the previous PE weight load in the
    same block with no intervening clobber (self-loading matmul or other PE
    instruction). The paired InstMatmult already has ldweights=False and
    simply reuses the stationary operand still resident in the PE array.
    Measured on HW: numerically exact, saves the redundant weight-load slot.
    """
    pe = mybir.EngineType.PE
    removed = 0
    for b in nc.m.functions[0].blocks:
        prev_key = None
        dead = set()
        for i in b.instructions:
            if i.engine != pe:
                continue
            if isinstance(i, mybir.InstLdweights):
                a = i.ins[0]
                key = (str(a.memref), a.offset, str(a.ap), str(a.dtype),
                       i.is_transpose, str(i.perf_mode), i.tile_position)
                si = i.sync_info
                has_sync = si is not None and (
                    len(si.on_wait) or len(si.on_update)
                )
                if key == prev_key and not has_sync:
                    dead.add(i.name)
                else:
                    prev_key = key
            elif isinstance(i, mybir.InstMatmult):
                if i.ldweights:
                    prev_key = None
            elif isinstance(i, mybir.InstEventSemaphore):
                pass
            else:
                prev_key = None
        if dead:
            insts = [x for x in b.instructions if x.name not in dead]
            b.instructions.clear()
            b.instructions.extend(insts)
            removed += len(dead)
    return removed


def _strip_unwaited_mm_updates(nc):
    """Every matmul increments the Tile PE progress semaphore (~26 ns of
    serialized EVT_SEM register traffic each), but only the increments whose
    cumulative count appears as a wait threshold are observable. Strip the
    rest and renumber all wait thresholds exactly.
    """
    f = nc.m.functions[0]
    pe = mybir.EngineType.PE

    # the PE progress semaphore: the one PE matmuls inc
    from collections import Counter
    sem_ids = Counter()
    for b in f.blocks:
        for i in b.instructions:
            if i.engine != pe or not isinstance(i, mybir.InstMatmult):
                continue
            si = i.sync_info
            if si is None:
                continue
            for u in si.on_update:
                if u.update_mode == "sem-inc" and u.update_value == 1:
                    sem_ids[u.id] += 1
    if not sem_ids:
        return 0
    SEM = sem_ids.most_common(1)[0][0]

    awaited = set()
    for b in f.blocks:
        for i in b.instructions:
            si = i.sync_info
            if si is None:
                continue
            for w in si.on_wait:
                if w.id == SEM:
                    assert w.wait_mode == "sem-ge-imm", w.wait_mode
                    awaited.add(w.wait_value)

    # walk PE stream in order: cumulative count, decide keep/strip
    cum = 0
    stripped_before = {}  # old count -> number stripped among first `count` incs
    nstrip = 0
    strip_insts = []
    for b in f.blocks:
        for i in b.instructions:
            if i.engine != pe:
                continue
            si = i.sync_info
            if si is None:
                continue
            ups = [u for u in si.on_update if u.id == SEM]
            if not ups:
                continue
            assert len(ups) == 1 and isinstance(i, mybir.InstMatmult), i.name
            cum += 1
            if cum not in awaited:
                nstrip += 1
                strip_insts.append(i)
            stripped_before[cum] = nstrip

    if not nstrip:
        return 0
    for i in strip_insts:
        si = i.sync_info
        si.on_update = [u for u in si.on_update if u.id != SEM]

    # renumber wait thresholds
    for b in f.blocks:
        for i in b.instructions:
            si = i.sync_info
            if si is None:
                continue
            for w in si.on_wait:
                if w.id == SEM:
                    w.wait_value = w.wait_value - stripped_before[w.wait_value]
    return nstrip


def _build_nc(reps=1):
    nc = bacc.Bacc("TRN2", target_bir_lowering=False)

    xt_d = nc.dram_tensor("xt", [G, 128, 2, N], BF16, kind="ExternalInput")
    xn_d = nc.dram_tensor("xn", [G, 128, 3, 2, 128], BF16, kind="ExternalInput")
    xnt_d = nc.dram_tensor("xnt", [G, 128, 128], BF16, kind="ExternalInput")
    at_d = nc.dram_tensor("at", [128, 3, N], BF16, kind="ExternalInput")
    att_d = nc.dram_tensor("att", [128, N], BF16, kind="ExternalInput")
    u_ds = [
        nc.dram_tensor(f"u{l}", [128, NJ, NFS[l - 1], 128], BF16,
                       kind="ExternalInput")
        for l in (1, 2, 3, 4)
    ]
    wdt_d = nc.dram_tensor("wdt", [128, NCC, 4, 128], BF16, kind="ExternalInput")
    bias_d = nc.dram_tensor("bias", [128, 4, NJ], F32, kind="ExternalInput")
    bdt_d = nc.dram_tensor("bdt", [128, 4], F32, kind="ExternalInput")
    # label-major output: y_d[g, p, lc, n] = y[g, n, lc*128+p]
    y_d = nc.dram_tensor("y", [G, 128, 4, N], F32, kind="ExternalOutput")

    with tile.TileContext(nc) as tc, ExitStack() as ctx:
        const = ctx.enter_context(tc.tile_pool(name="const", bufs=1))
        xpool = ctx.enter_context(tc.tile_pool(name="xpool", bufs=2))
        hpool = ctx.enter_context(tc.tile_pool(name="hpool", bufs=2))
        rpool = ctx.enter_context(tc.tile_pool(name="rpool", bufs=2))
        ypool = ctx.enter_context(tc.tile_pool(name="ypool", bufs=2))
        ps_gso = ctx.enter_context(tc.tile_pool(name="ps_gso", bufs=2, space="PSUM"))
        ps_chan = ctx.enter_context(tc.tile_pool(name="ps_chan", bufs=5, space="PSUM"))
        ps_tr = ctx.enter_context(tc.tile_pool(name="ps_tr", bufs=1, space="PSUM"))

        # ---- resident constants (weights loaded once, never re-streamed).
        # Only what layer 1 needs is DMA'd up front; the bulk U2-4 loads are
        # issued inside pair 0's body so they don't head-of-line-block the
        # first pair's inputs on the SP DMA queue. ----
        ident = const.tile([128, 128], BF16)
        make_identity(nc, ident)
        u_sbs = [
            const.tile([128, NJ, NFS[l - 1], 128], BF16, tag=f"u{l}",
                       name=f"u_sb{l}")
            for l in (1, 2, 3, 4)
        ]
        at_sb = const.tile([128, 3, N], BF16)
        nc.scalar.dma_start(out=at_sb[:], in_=at_d[:])
        att_sb = const.tile([128, N], BF16)
        nc.scalar.dma_start(out=att_sb[:], in_=att_d[:])
        # u1/bias/wdt/bdt are DMA'd from inside pair 0's body, after its
        # input loads, ordered by first use so the first GSO isn't queued
        # behind them.
        wdt_sb = const.tile([128, NCC, 4, 128], BF16)
        bias_sb = const.tile([128, 4, NJ], F32)
        bdt_sb = const.tile([128, 4], F32)

        def gso_layer1(g, xn_g, xnt_g, xa_out):
            # Layer-1 GSO (2 feature chunks): 3 full m-tiles + packed tail.
            # The tails of both fc chunks sit at partition offsets 0/32 of
            # xnt, so the two row-tiled tail matmuls co-execute on the PE.
            pss = {}
            for fc in (0, 1):
                ps = ps_gso.tile([128, N], F32, tag="gso", name=f"psg1_{g}_{fc}")
                for mt in range(3):
                    nc.tensor.matmul(
                        ps[:],
                        xn_g[:, mt, fc, :],
                        at_sb[:, mt, :],
                        start=(mt == 0),
                        stop=False,
                    )
                pss[fc] = ps
            for fc in (0, 1):
                nc.tensor.matmul(
                    pss[fc][:],
                    xnt_g[32 * fc : 32 * fc + 16, :],
                    att_sb[32 * fc : 32 * fc + 16, :],
                    start=False,
                    stop=True,
                    tile_position=(32 * fc, 0),
                )
            for fc in (0, 1):
                nc.vector.tensor_copy(xa_out[:, fc, :], pss[fc][:])

        def gso_layer2(g, li, hn_g, hnt_g, xa_out):
            # Layers 2-4 GSO: 3 full 128-node m-tiles + a packed tail step.
            # The tail (nodes 384:399, 16 rows) for both fc of a half sits at
            # 32-aligned partition offsets in hn_tail, so the two row-tiled
            # matmuls occupy different PE row groups and co-execute.
            for half in (0, 1):
                fcs = (2 * half, 2 * half + 1)
                pss = {}
                for fc in fcs:
                    ps = ps_gso.tile([128, N], F32, tag="gso",
                                     name=f"psg_{g}_{li}_{fc}")
                    for mt in range(3):
                        nc.tensor.matmul(
                            ps[:],
                            hn_g[:, fc, mt, :],
                            at_sb[:, mt, :],
                            start=(mt == 0),
                            stop=False,
                        )
                    pss[fc] = ps
                for fc in fcs:
                    nc.tensor.matmul(
                        pss[fc][:],
                        hnt_g[32 * fc : 32 * fc + 16, :],
                        att_sb[32 * fc : 32 * fc + 16, :],
                        start=False,
                        stop=True,
                        tile_position=(32 * fc, 0),
                    )
                for fc in fcs:
                    nc.vector.tensor_copy(xa_out[:, fc, :], pss[fc][:])

        def chan_layer(gs, li, u_sb, nf, moving, hT_out):
            # Z.T[jj] = sum_f U[f,jj].T @ xcatT[f] ; relu+bias ; sum K stacks.
            # f outer / graph inner: each stationary U block serves both
            # graphs of the pair back-to-back.
            for cc in range(NCC):
                rk = {g: [] for g in gs}
                for k in range(K):
                    jj = k * NCC + cc
                    pss = {
                        g: ps_chan.tile([128, 512], F32, tag="chan",
                                        name=f"psc_{g}_{jj}")
                        for g in gs
                    }
                    for f in range(nf):
                        for g in gs:
                            nc.tensor.matmul(
                                pss[g][:, :N],
                                u_sb[:, jj, f, :],
                                moving(g, f),
                                start=(f == 0),
                                stop=(f == nf - 1),
                            )
                    for g in gs:
                        r = rpool.tile([128, N], BF16, tag=f"r{g % 2}_{k}",
                                       name=f"r_{g}_{k}")
                        nc.scalar.activation(
                            r[:], pss[g][:, :N], RELU,
                            bias=bias_sb[:, li, jj : jj + 1],
                        )
                        rk[g].append(r)
                for g in gs:
                    nc.vector.tensor_add(hT_out[g][:, cc, :N], rk[g][0][:], rk[g][1][:])
                    nc.vector.tensor_add(
                        hT_out[g][:, cc, :N], hT_out[g][:, cc, :N], rk[g][2][:]
                    )

        def transpose_h(hT, h_node, h_tail):
            # hT [128, NCC, 512] -> h_node [128, NCC, 3, 128] (m-tiles 0-2)
            # plus h_tail [128, 128]: partition 32*cc+i = node 384+i of chunk
            # cc (i>=16 is junk from hT cols 400:415, never read).
            # The 12 full 128x128 transposes go through the DMA xbar
            # transpose (idle DMA engines, ~112 ns each) instead of the PE;
            # only the 4 cheap 16-col tail transposes stay on the PE (they
            # need the 32-aligned partition packing for GSO co-execution).
            ps = ps_tr.tile([128, 128], BF16, tag="tr")
            for cc in range(NCC):
                nc.tensor.transpose(
                    ps[32 * cc : 32 * cc + 16, :],
                    hT[:, cc, 384:400],
                    ident[:],
                    tile_position=(0, 32 * cc),
                )
            nc.vector.tensor_copy(h_tail[:], ps[:])
            for cc in range(NCC):
                # one blockwise xbar transpose per chunk:
                # h_node[:, cc, mt, :] = hT[:, cc, mt*128:(mt+1)*128].T
                # alternating dispatch queues (SP / Activation)
                eng = nc.sync if cc % 2 == 0 else nc.scalar
                eng.dma_start_transpose(
                    out=h_node[:, cc, :, :],
                    in_=hT[:, cc, 0:384],
                )

        rep_ctx = tc.For_i(0, reps, 1) if reps > 1 else nullcontext()
        with rep_ctx:
         for pp in range(G // 2):
            gs = (2 * pp, 2 * pp + 1)

            # ---------- layer 1 ----------
            xt_sb, xn_sb, xnt_sb, xa1, hT, hn, hnt = {}, {}, {}, {}, {}, {}, {}
            for g in gs:
                # xn before xt: the first GSO needs only at+xn; xt is first
                # read by the chan layer a few us later.
                xn_sb[g] = xpool.tile([128, 3, 2, 128], BF16, tag=f"xn{g % 2}",
                                      name=f"xn_g{g}")
                nc.sync.dma_start(out=xn_sb[g][:], in_=xn_d[g])
                xnt_sb[g] = xpool.tile([128, 128], BF16, tag=f"xnt{g % 2}",
                                       name=f"xnt_g{g}")
                nc.sync.dma_start(out=xnt_sb[g][:], in_=xnt_d[g])
                xt_sb[g] = xpool.tile([128, 2, N], BF16, tag=f"xt{g % 2}",
                                      name=f"xt_g{g}")
                nc.sync.dma_start(out=xt_sb[g][:], in_=xt_d[g])
                xa1[g] = xpool.tile([128, 2, N], BF16, tag=f"xa_{g % 2}",
                                    name=f"xa1_g{g}")
                gso_layer1(g, xn_sb[g], xnt_sb[g], xa1[g])
                hT[g] = hpool.tile([128, NCC, 512], BF16, tag=f"hT{g % 2}",
                                   name=f"hT1_g{g}")
            if pp == 0:
                nc.sync.dma_start(out=u_sbs[0][:], in_=u_ds[0][:])
                nc.sync.dma_start(out=bias_sb[:], in_=bias_d[:])
            chan_layer(
                gs, 0, u_sbs[0], 4,
                lambda g, f: xt_sb[g][:, f, :] if f < 2 else xa1[g][:, f - 2, :],
                hT,
            )
            if pp == 0:
                # Bulk loads issued after pair 0's L1 so they don't
                # head-of-line block the first pair's inputs on the DMA queue.
                for l in (1, 2, 3):
                    nc.sync.dma_start(out=u_sbs[l][:], in_=u_ds[l][:])
                nc.sync.dma_start(out=wdt_sb[:], in_=wdt_d[:])
                nc.sync.dma_start(out=bdt_sb[:], in_=bdt_d[:])
            for g in gs:
                hn[g] = hpool.tile([128, NCC, 3, 128], BF16, tag=f"hn{g % 2}",
                                   name=f"hn1_g{g}")
                hnt[g] = hpool.tile([128, 128], BF16, tag=f"hnt{g % 2}",
                                    name=f"hnt1_g{g}")
                transpose_h(hT[g], hn[g], hnt[g])

            # ---------- layers 2..4 ----------
            for li, u_sb in zip((1, 2, 3), u_sbs[1:]):
                xa, hT_new, hn_new, hnt_new = {}, {}, {}, {}
                for g in gs:
                    xa[g] = xpool.tile([128, NCC, N], BF16, tag=f"xa_{g % 2}",
                                       name=f"xa_g{g}_l{li}")
                    gso_layer2(g, li, hn[g], hnt[g], xa[g])
                    hT_new[g] = hpool.tile([128, NCC, 512], BF16, tag=f"hT{g % 2}",
                                           name=f"hT_g{g}_l{li}")
                chan_layer(
                    gs, li, u_sb, 8,
                    lambda g, f: hT[g][:, f, :N] if f < NCC else xa[g][:, f - NCC, :],
                    hT_new,
                )
                if li < 3:
                    for g in gs:
                        hn_new[g] = hpool.tile([128, NCC, 3, 128], BF16,
                                               tag=f"hn{g % 2}", name=f"hn_g{g}_l{li}")
                        hnt_new[g] = hpool.tile([128, 128], BF16,
                                                tag=f"hnt{g % 2}",
                                                name=f"hnt_g{g}_l{li}")
                        transpose_h(hT_new[g], hn_new[g], hnt_new[g])
                    hn, hnt = hn_new, hnt_new
                hT = hT_new

            # ---------- dense head (label-major) ----------
            # yT[lc*128+p, n] = sum_c Wd.T[l, c] h[n, c] / 3 + bd[l].
            # Stationary = Wd chunk (shared by both graphs of the pair and
            # LDW-deduped), moving = hT chunk (400 wide vs 480 node-major).
            # Host un-packs the label-major layout.
            y_sb = {}
            for g in gs:
                y_sb[g] = ypool.tile([128, 4, N], F32, tag=f"y{g % 2}",
                                     name=f"y_{g}")
            for lc in range(4):
                pss = {}
                for cc in range(NCC):
                    for g in gs:
                        if cc == 0:
                            pss[g] = ps_chan.tile([128, 512], F32, tag="chan",
                                                  name=f"psy_{g}_{lc}")
                        nc.tensor.matmul(
                            pss[g][:, :N],
                            wdt_sb[:, cc, lc, :],
                            hT[g][:, cc, :N],
                            start=(cc == 0),
                            stop=(cc == NCC - 1),
                        )
                for g in gs:
                    nc.vector.tensor_scalar_add(
                        y_sb[g][:, lc, :], pss[g][:, :N],
                        bdt_sb[:, lc : lc + 1],
                    )
            for g in gs:
                nc.scalar.dma_start(out=y_d[g], in_=y_sb[g][:])

    nc.compile()
    if reps == 1:
        # straight-line code only: the semaphore renumbering in the strip
        # pass assumes each block executes once
        _dedupe_ldweights(nc)
        _strip_unwaited_mm_updates(nc)
    return nc


def _pack_inputs(x, a, Ws, Vs, bs, Wd, bd):
    """Host-side packing into the per-core DMA-friendly layouts (bf16)."""
    import ml_dtypes

    BF = ml_dtypes.bfloat16
    x = np.asarray(x, np.float32)
    a = np.asarray(a, np.float32)

    # aT m-tiles 0-2: [p, mt, n] = a[n, mt*128+p] (tail nodes live in att)
    at_pack = np.ascontiguousarray(
        a.T[: 3 * 128, :].reshape(3, 128, N).transpose(1, 0, 2)
    ).astype(BF)

    # tail replica: att[32*cc + i, n] = a[n, 384 + i] for i < 16, all 4 cc
    att_pack = np.zeros((128, N), np.float32)
    for cc in range(NCC):
        att_pack[32 * cc : 32 * cc + 16, :] = a.T[384:400, :]
    att_pack = att_pack.astype(BF)

    # U layouts: [128, NJ, nf, 128]; u[p, jj, f, c] = U[f*128+p, jj*128+c]
    # U1: rows [V1 pad 256 ; W1 pad 256], cols k-major (k*512 + c)
    U1 = np.zeros((512, K * C), np.float32)
    for k in range(K):
        U1[:F, k * C : (k + 1) * C] = Vs[0][k]
        U1[FP : FP + F, k * C : (k + 1) * C] = Ws[0][k]
    u_packs = [
        np.ascontiguousarray(
            U1.reshape(4, 128, NJ, 128).transpose(1, 2, 0, 3)
        ).astype(BF)
    ]
    for l in range(1, 4):
        U = np.empty((2 * C, K * C), np.float32)
        for k in range(K):
            U[:C, k * C : (k + 1) * C] = Vs[l][k] / 3.0
            U[C:, k * C : (k + 1) * C] = Ws[l][k] / 3.0
        u_packs.append(
            np.ascontiguousarray(
                U.reshape(8, 128, NJ, 128).transpose(1, 2, 0, 3)
            ).astype(BF)
        )

    # wdt[p, cc, lc, j] = Wd[cc*128+p, lc*128+j] / 3, zero-padded past L
    wd3 = np.asarray(Wd, np.float32) / 3.0
    wdt_pack = np.zeros((128, NCC, 4, 128), np.float32)
    for cc in range(NCC):
        for lc in range(4):
            w = min(128, L - lc * 128)
            wdt_pack[:, cc, lc, :w] = wd3[cc * 128 : (cc + 1) * 128,
                                          lc * 128 : lc * 128 + w]
    wdt_pack = wdt_pack.astype(BF)

    bias_pack = np.zeros((128, 4, NJ), np.float32)
    for li in range(4):
        for jj in range(NJ):
            k, cc = divmod(jj, NCC)
            bias_pack[:, li, jj] = bs[li][k, cc * 128 : (cc + 1) * 128]

    # bdt[p, lc] = bd[lc*128+p] (zero past L)
    bdt_pack = np.zeros((128, 4), np.float32)
    bdf = np.asarray(bd, np.float32)
    for lc in range(4):
        w = min(128, L - lc * 128)
        bdt_pack[:w, lc] = bdf[lc * 128 : lc * 128 + w]

    in_maps = []
    for c in range(NCORES):
        xs = x[c * G : (c + 1) * G]  # (G, 400, 240)
        x_pad = np.zeros((G, NP, FP), np.float32)
        x_pad[:, :N, :F] = xs
        xn_pack = np.ascontiguousarray(
            x_pad[:, : 3 * 128, :].reshape(G, 3, 128, 2, 128)
            .transpose(0, 2, 1, 3, 4)
        ).astype(BF)
        # xnt[g, 32*fc + i, c] = x[g, 384+i, fc*128+c] for fc in (0,1)
        xnt_pack = np.zeros((G, 128, 128), np.float32)
        for fc in (0, 1):
            xnt_pack[:, 32 * fc : 32 * fc + 16, :] = x_pad[
                :, 384:400, fc * 128 : (fc + 1) * 128
            ]
        xnt_pack = xnt_pack.astype(BF)
        xt_pack = np.ascontiguousarray(
            x_pad[:, :N, :].transpose(0, 2, 1).reshape(G, 2, 128, N).transpose(0, 2, 1, 3)
        ).astype(BF)
        in_maps.append(
            {
                "xt": xt_pack,
                "xn": xn_pack,
                "xnt": xnt_pack,
                "at": at_pack,
                "att": att_pack,
                "u1": u_packs[0],
                "u2": u_packs[1],
                "u3": u_packs[2],
                "u4": u_packs[3],
                "wdt": wdt_pack,
                "bias": bias_pack,
                "bdt": bdt_pack,
            }
        )
    return in_maps


_NC_CACHE = {}


def _get_nc(reps=1):
    key = f"nc{reps}"
    if key not in _NC_CACHE:
        _NC_CACHE[key] = _build_nc(reps)
    return _NC_CACHE[key]


def _get_runner():
    """Compile-once jitted SPMD executor (mirrors bass2jax.run_bass_via_pjrt's
    multi-core branch) so repeated kernel() calls skip recompilation."""
    if "runner" in _NC_CACHE:
        return _NC_CACHE["runner"]
    import jax
    import jax.numpy as jnp
    from jax.sharding import Mesh, PartitionSpec, NamedSharding
    from jax.experimental.shard_map import shard_map
    from concourse import bass2jax

    nc = _get_nc()
    bass2jax.install_neuronx_cc_hook()
    partition_name = nc.partition_id_tensor.name if nc.partition_id_tensor else None
    in_names, out_names, out_avals, zero_shapes = [], [], [], []
    for alloc in nc.m.functions[0].allocations:
        if not isinstance(alloc, mybir.MemoryLocationSet):
            continue
        name = alloc.memorylocations[0].name
        if alloc.kind == "ExternalInput":
            if name != partition_name:
                in_names.append(name)
        elif alloc.kind == "ExternalOutput":
            out_names.append(name)
            shape = tuple(alloc.tensor_shape)
            dtype = mybir.dt.np(alloc.dtype)
            out_avals.append(jax.core.ShapedArray(shape, dtype))
            zero_shapes.append((shape, dtype))
    n_params = len(in_names)
    n_outs = len(out_avals)
    all_names = list(in_names) + list(out_names)
    if partition_name is not None:
        all_names.append(partition_name)

    def _body(*args):
        operands = list(args)
        if partition_name is not None:
            operands.append(bass2jax.partition_id_tensor())
        outs = bass2jax._bass_exec_p.bind(
            *operands,
            out_avals=tuple(out_avals),
            in_names=tuple(all_names),
            out_names=tuple(out_names),
            lowering_input_output_aliases=(),
            sim_require_finite=True,
            sim_require_nnan=True,
            nc=nc,
        )
        return tuple(outs)

    devices = jax.devices()[:NCORES]
    mesh = Mesh(np.asarray(devices), ("core",))
    sharded = jax.jit(
        shard_map(
            _body,
            mesh=mesh,
            in_specs=(PartitionSpec("core"),) * (n_params + n_outs),
            out_specs=(PartitionSpec("core"),) * n_outs,
            check_rep=False,
        ),
        donate_argnums=tuple(range(n_params, n_params + n_outs)),
        keep_unused=True,
    )
    sh = NamedSharding(mesh, PartitionSpec("core"))
    make_zeros = jax.jit(
        lambda: tuple(
            jnp.zeros((NCORES * s[0], *s[1:]), d) for s, d in zero_shapes
        ),
        out_shardings=(sh,) * n_outs,
    )

    def run(in_maps):
        concat = [
            np.concatenate([np.asarray(m[name]) for m in in_maps], axis=0)
            for name in in_names
        ]
        dev_in = [jax.device_put(a, sh) for a in concat]
        outs = sharded(*dev_in, *make_zeros())
        oi = out_names.index("y")
        # y_raw[b, p, lc, n] = y[b, n, lc*128+p]
        y_raw = np.asarray(outs[oi])  # (B, 128, 4, N)
        y = np.ascontiguousarray(y_raw.transpose(0, 3, 2, 1)).reshape(B, N, 512)
        return y[:, :, :L]

    _NC_CACHE["runner"] = run
    return run


def kernel(
    x, a, W1, V1, b1, W2, V2, b2, W3, V3, b3, W4, V4, b4, Wd, bd
) -> np.ndarray:
    in_maps = _pack_inputs(
        x,
        a,
        [np.asarray(W, np.float32) for W in (W1, W2, W3, W4)],
        [np.asarray(V, np.float32) for V in (V1, V2, V3, V4)],
        [np.asarray(b, np.float32) for b in (b1, b2, b3, b4)],
        Wd,
        bd,
    )
    return _get_runner()(in_maps)

